# revision 36
# baseline (speedup 1.0000x reference)
"""BlockWiseEmbedding kernel for 8 Trainium2 NeuronCores.

Strategy (data-parallel tokens, replicated tables, all-bf16 datapath):
  - Host: route each token to its block via block_assignment/local_assignment,
    dedup rows per block (np.unique) and deal them evenly across the 8 cores.
    Tables and transformer weights are converted to bf16 on the host (block0
    zero-padded from 64 to 128 cols so its gather rows satisfy the 256B
    elem-size rule); the well-conditioned f32 accumulate happens in PSUM.
  - Device (identical SPMD program on all 8 cores):
      * SWDGE dma_gather instructions fetch table rows; each block owns its
        own SWDGE queue so descriptor generation runs concurrently on
        different Q7 core pairs, and blocks are split into chunk parts so
        data streams to the PE as desc-gen progresses (at most 8 gather
        instructions: the SWDGE semaphore pool has 8 queue-locked slots)
      * PE transposes (bf16, 1 cycle/row) flip gathered [token, s] tiles
        into matmul-ready layout; DVE/ACT copy them PSUM->SBUF
      * bf16 matmuls accumulate [128 tok x 512] in PSUM f32; ~100 PE warmup
        transposes keep the p-state ramp alive through the ~12us GPSIMD
        library reload that gates the first gather
      * results are copied to bf16 SBUF (DVE/ACT alternating) and DMA'd out
        once per block (the tail block ships its last chunk separately)
  - Host: convert bf16 outputs to f32 and scatter back to token order.
"""

import os
import sys

import numpy as np

for _p in ("/opt/trn_rl_repo", "/root/.axon_site/_ro/trn_rl_repo"):
    if os.path.isdir(_p) and _p not in sys.path:
        sys.path.append(_p)

import ml_dtypes

BF16 = np.dtype(ml_dtypes.bfloat16)

N_CORES = 8
OUT_DIM = 512
N_BLOCKS = 4

TRACE = False
# dummy PE transposes issued while the GPSIMD library loads and the first
# gather descriptors are generated, keeping the PE pipeline ramped and the
# p-state ramp alive until real matmul data arrives
PE_WARMUP = 100

LAST_EXEC_NS = None
LAST_RESULTS = None

_CACHE = {}


def _cdiv(a, b):
    return -(-a // b)


def _pad_cols(s):
    """bf16 gather rows must be a multiple of 256 bytes -> >=128 cols."""
    return max(128, _cdiv(s, 128) * 128)


def _build_program(sizes, table_rows, nb16, out_dim):
    import concourse.mybir as mybir
    from concourse import bacc, tile
    from concourse._compat import get_trn_type
    from concourse.library_config import mlp

    f32 = mybir.dt.float32
    i16 = mybir.dt.int16
    bf16 = mybir.dt.bfloat16
    nB = len(sizes)
    spad = [_pad_cols(s) for s in sizes]
    nk = [sp // 128 for sp in spad]
    C = [_cdiv(n, 128) for n in nb16]

    # idx columns (16 tokens per column, baseline layout)
    ioffs = [0]
    for n in nb16:
        ioffs.append(ioffs[-1] + n)
    totcols = ioffs[-1] // 16

    # padded output layout: each block's segment is C*128 rows
    ooffs = [0]
    for c in C:
        ooffs.append(ooffs[-1] + c * 128)
    totpad = ooffs[-1]

    # process big blocks first: their matmul chains are the deepest
    border = sorted(range(nB), key=lambda b: -spad[b])
    # trans chunk offsets in the packed weight tensor, in border order
    koff = {}
    kk = 0
    for b in border:
        koff[b] = kk
        kk += nk[b]
    tot_nk = kk

    nc = bacc.Bacc(
        get_trn_type() or "TRN2", target_bir_lowering=False, num_swdge_queues=4
    )
    tabs = [
        nc.dram_tensor(f"block{b}", [table_rows[b], spad[b]], bf16, kind="ExternalInput")
        for b in range(nB)
    ]
    # cpk packs idx columns + bf16 identity (for PE warmup) in one DMA
    cpk = nc.dram_tensor("cpk", [128, totcols + 128], i16, kind="ExternalInput")
    # trp packs all transformer weights [p, sum nk, out_dim] in one DMA
    trp = nc.dram_tensor("trp", [128, tot_nk, out_dim], bf16, kind="ExternalInput")
    out = nc.dram_tensor("out", [totpad, out_dim], bf16, kind="ExternalOutput")

    nc.gpsimd.load_library(mlp)

    # engine-balance for PSUM->SBUF output copies
    load = {"v": 0.0, "s": 0.0}

    def copy_psum(dst, src, elems):
        if load["v"] <= load["s"]:
            nc.vector.tensor_copy(dst, src)
            load["v"] += elems
        else:
            nc.scalar.copy(dst, src)
            load["s"] += elems * 1.7

    from concourse.bass import IndirectOffsetOnAxis

    with tile.TileContext(nc) as tc:
        with (
            tc.tile_pool(name="const", bufs=1) as cpool,
            tc.tile_pool(name="gath", bufs=1) as gpool,
            tc.tile_pool(name="et", bufs=10) as epool,
            tc.tile_pool(name="ot", bufs=1) as opool,
            tc.tile_pool(name="pt", bufs=4, space="PSUM") as ptpool,
            tc.tile_pool(name="po", bufs=3, space="PSUM") as ppool,
            tc.tile_pool(name="warm", bufs=1, space="PSUM") as wpool,
        ):
            cpk_sb = cpool.tile([128, totcols + 128], i16)
            nc.sync.dma_start(cpk_sb[:], cpk[:, :])
            idx_sb = cpk_sb[:, :totcols]
            ident = cpk_sb[:, totcols : totcols + 128].bitcast(bf16)
            tr_sb = cpool.tile([128, tot_nk, out_dim], bf16, tag="trp")
            nc.sync.dma_start(tr_sb[:], trp[:, :, :])

            # keep the PE hot (and its p-state ramped) from the identity DMA
            # until real matmul data arrives
            if PE_WARMUP:
                warm = wpool.tile([128, 128], bf16, tag="warm")
                for _ in range(PE_WARMUP):
                    nc.tensor.transpose(warm[:], ident, ident)

            # chunked gather splits stream data to the PE as descriptor
            # generation progresses; the three biggest blocks own SWDGE
            # rings 1-3 (desc-gen on dedicated Q7 pairs). The SWDGE sem pool
            # has 8 slots, round-robined over Pool DMA instructions and each
            # locked to one queue -- so at most 8 gather instructions total:
            # 3 parts for the deepest block, 2 each for the next two, and
            # the tail block whole on ring 0
            big3 = [b for b in border[:3] if nb16[b]]
            tail_blocks = [b for b in border[3:] if nb16[b]]
            qmap = {b: i + 1 for i, b in enumerate(big3)}

            def splitn(c, n):
                if c <= 1 or n == 1:
                    return [(0, c)]
                if n >= 3 and c > 2:
                    return [(0, 1), (1, 2), (2, c)]
                return [(0, 1), (1, c)]

            gather_parts = {}
            for i, b in enumerate(big3):
                gather_parts[b] = splitn(C[b], 3 if i == 0 else 2)
            for b in tail_blocks:
                gather_parts[b] = splitn(C[b], 1)

            g_sb = {}  # (b, part_i) -> tile
            for b in border:
                if not nb16[b]:
                    continue
                for pi, (lo, hi) in enumerate(gather_parts[b]):
                    g = gpool.tile(
                        [128, hi - lo, spad[b]], bf16,
                        tag=f"g{b}p{pi}", name=f"g{b}p{pi}",
                    )
                    if hi * 128 > nb16[b]:
                        # zero the partial last chunk so ungathered token
                        # slots stay finite downstream
                        nc.vector.memset(g[:, hi - lo - 1, :].bitcast(f32), 0.0)
                    g_sb[(b, pi)] = g

            def emit_gather(b, pi, q):
                lo, hi = gather_parts[b][pi]
                sp = spad[b]
                n_idx = min(nb16[b], hi * 128) - lo * 128
                nc.gpsimd.dma_gather(
                    g_sb[(b, pi)][:, :, :],
                    tabs[b][:, :],
                    idx_sb[
                        :,
                        ioffs[b] // 16 + lo * 8 : ioffs[b] // 16 + lo * 8 + n_idx // 16,
                    ],
                    n_idx,
                    n_idx,
                    sp,
                    queue_num=q,
                )

            # dispatch: the big blocks' parts round-robin so the three rings
            # desc-gen concurrently, then the tail block's parts spread
            # across the same rings
            for part in range(3):
                for b in big3:
                    if len(gather_parts[b]) > part:
                        emit_gather(b, part, qmap[b])
            # tail block parts share ring 0 (sems are locked to one queue);
            # its desc-gen runs synchronously on the Pool dispatcher, which
            # has nothing left to dispatch by then
            for b in tail_blocks:
                for pi in range(len(gather_parts[b])):
                    emit_gather(b, pi, 0)

            def g_chunk(b, m, k):
                """gathered [128 tok, 128 s] slice for token-chunk m."""
                for pi, (lo, hi) in enumerate(gather_parts[b]):
                    if lo <= m < hi:
                        return g_sb[(b, pi)][:, m - lo, k * 128 : (k + 1) * 128]
                raise AssertionError

            # PE transposes + bf16 matmuls, round-robin chunks across the
            # async blocks (their data lands first), queue-0 blocks last
            ot_sb = {}
            for b in border:
                if nb16[b]:
                    ot_sb[b] = opool.tile(
                        [128, C[b], out_dim], bf16, tag=f"ot{b}", name=f"ot{b}"
                    )
            # order matmul chunks by estimated data arrival: all chunk0s
            # first (each ring's part A lands ~together), then the deepest
            # block's remaining chunks (its ring streams parts B/C first),
            # then the other rings' remainders, tail block last
            sched = []
            for b in big3:
                sched.append((b, 0))
            for b in big3:
                for m in range(1, C[b]):
                    sched.append((b, m))
            for b in tail_blocks:
                for m in range(C[b]):
                    sched.append((b, m))
            for b, m in sched:
                ets = []
                for k in range(nk[b]):
                    pt = ptpool.tile([128, 128], bf16, tag="pt")
                    nc.tensor.transpose(pt[:, :], g_chunk(b, m, k), ident)
                    et = epool.tile([128, 128], bf16, tag="et")
                    copy_psum(et[:, :], pt[:, :], 128 * 128)
                    ets.append(et)
                po = ppool.tile([128, out_dim], f32, tag="po")
                for k, et in enumerate(ets):
                    nc.tensor.matmul(
                        po[:, :],
                        et[:, :],
                        tr_sb[:, koff[b] + k, :],
                        start=(k == 0),
                        stop=(k == nk[b] - 1),
                    )
                # split the output copy across both copy engines: halves
                # the PSUM->SBUF latency on the tail-chunk critical chain
                oh = out_dim // 2
                nc.vector.tensor_copy(ot_sb[b][:, m, :oh], po[:, :oh])
                nc.scalar.copy(ot_sb[b][:, m, oh:], po[:, oh:])
                if b in tail_blocks and C[b] > 1:
                    # the last-scheduled block gates the kernel end: ship its
                    # first chunks early and only the final 128 rows last
                    if m == C[b] - 2:
                        nc.sync.dma_start(
                            out[ooffs[b] : ooffs[b] + (C[b] - 1) * 128, :].rearrange(
                                "(m p) n -> p m n", p=128
                            ),
                            ot_sb[b][:, : C[b] - 1, :],
                        )
                    elif m == C[b] - 1:
                        nc.sync.dma_start(
                            out[
                                ooffs[b] + (C[b] - 1) * 128 : ooffs[b] + C[b] * 128, :
                            ].rearrange("(m p) n -> p m n", p=128),
                            ot_sb[b][:, C[b] - 1 :, :],
                        )
                elif m == C[b] - 1:
                    # whole block computed: one output DMA per block
                    nc.sync.dma_start(
                        out[ooffs[b] : ooffs[b] + C[b] * 128, :].rearrange(
                            "(m p) n -> p m n", p=128
                        ),
                        ot_sb[b][:, :, :],
                    )

    nc.compile()
    return nc, ooffs, totpad


def _route(src, block_assignment, local_assignment, table_rows):
    """Host-side token routing with row dedup. Each block's referenced table
    rows are deduplicated (np.unique, so per-core gather indices are sorted
    ascending -> better HBM locality) and dealt evenly across cores. Returns
    per-core index buffers plus bookkeeping to reassemble outputs."""
    src_f = np.asarray(src).reshape(-1)
    ba = np.asarray(block_assignment)[src_f]
    la = np.asarray(local_assignment)[src_f]

    nb = [0] * N_BLOCKS
    nb16 = [0] * N_BLOCKS
    binfo = []
    for b in range(N_BLOCKS):
        toks = np.where(ba == b)[0]
        rows = np.clip(la[toks], 0, table_rows[b] - 1)
        urows, inv = np.unique(rows, return_inverse=True)
        binfo.append((toks, inv, urows))
        nb[b] = int(_cdiv(urows.size, N_CORES))
        nb16[b] = _cdiv(nb[b], 16) * 16

    ioffs = [0]
    for n in nb16:
        ioffs.append(ioffs[-1] + n)
    totcols = ioffs[-1] // 16

    idx_bufs = np.zeros((N_CORES, 128, totcols), dtype=np.int16)
    for b in range(N_BLOCKS):
        toks, inv, urows = binfo[b]
        if urows.size == 0:
            continue
        for c in range(N_CORES):
            lo = c * nb[b]
            hi = min(urows.size, lo + nb[b])
            if hi <= lo:
                continue
            pad = np.zeros((nb16[b],), dtype=np.int16)
            pad[: hi - lo] = urows[lo:hi].astype(np.int16)
            # index j lives at [j % 16, j // 16], segment starts at column
            # ioffs[b] // 16; the 16-partition block is replicated to all 128
            # partitions (each Q7 core pair reads its own copy)
            wrapped = pad.reshape(-1, 16).T
            idx_bufs[c, :, ioffs[b] // 16 : ioffs[b] // 16 + nb16[b] // 16] = np.tile(
                wrapped, (8, 1)
            )
    return idx_bufs, binfo, tuple(nb), tuple(nb16)


def _host_tensors(blocks, trans, sizes, nb16):
    """bf16-convert tables and weights into the packed device layouts."""
    nB = len(sizes)
    spad = [_pad_cols(s) for s in sizes]
    nk = [sp // 128 for sp in spad]
    border = sorted(range(nB), key=lambda b: -spad[b])
    tot_nk = sum(nk)

    tabs = []
    for b in range(nB):
        if spad[b] != sizes[b]:
            t = np.zeros((blocks[b].shape[0], spad[b]), dtype=BF16)
            t[:, : sizes[b]] = blocks[b].astype(BF16)
        else:
            t = np.ascontiguousarray(blocks[b].astype(BF16))
        tabs.append(t)

    trp = np.zeros((128, tot_nk, OUT_DIM), dtype=BF16)
    kk = 0
    koff = {}
    for b in border:
        koff[b] = kk
        tr = trans[b].astype(BF16)
        for k in range(nk[b]):
            rows = tr[k * 128 : (k + 1) * 128]
            trp[: rows.shape[0], kk + k, :] = rows
        kk += nk[b]

    ident = np.ascontiguousarray(np.eye(128, dtype=BF16)).view(np.int16)
    return tabs, trp, ident


def _ensure_ntff_hook():
    """Register the axon NTFF profiling hook if the image's antenv lacks it."""
    try:
        from antenv.axon_hooks import get_axon_ntff_profile_hook  # noqa: F401

        return
    except ImportError:
        pass
    import types

    mod = types.ModuleType("antenv.axon_hooks")
    holder = {"h": None}
    mod.set_axon_ntff_profile_hook = lambda h: holder.__setitem__("h", h)
    mod.get_axon_ntff_profile_hook = lambda: holder["h"]
    sys.modules["antenv.axon_hooks"] = mod
    try:
        if "/root/.axon_site" not in sys.path:
            sys.path.append("/root/.axon_site")
        from trn_agent_boot.trn_boot import _ntff_profile_via_ctypes

        so = "/opt/axon/libaxon_pjrt.so"
        if os.path.exists(so):
            h = _ntff_profile_via_ctypes(so)
            if h is not None:
                mod.set_axon_ntff_profile_hook(h)
    except Exception:
        pass


def kernel(
    src,
    block_assignment,
    local_assignment,
    block0,
    block1,
    block2,
    block3,
    trans0,
    trans1,
    trans2,
    trans3,
):
    global LAST_EXEC_NS, LAST_RESULTS
    from concourse.bass_utils import run_bass_kernel_spmd

    blocks = [np.asarray(x, dtype=np.float32) for x in (block0, block1, block2, block3)]
    trans = [np.asarray(x, dtype=np.float32) for x in (trans0, trans1, trans2, trans3)]
    sizes = [b.shape[1] for b in blocks]
    table_rows = [b.shape[0] for b in blocks]
    src = np.asarray(src)

    idx_bufs, binfo, nb, nb16 = _route(
        src, block_assignment, local_assignment, table_rows
    )

    key = (tuple(sizes), tuple(table_rows), nb16)
    if key not in _CACHE:
        _CACHE[key] = _build_program(sizes, table_rows, list(nb16), OUT_DIM)
    nc, ooffs, totpad = _CACHE[key]

    tabs, trp, ident = _host_tensors(blocks, trans, sizes, nb16)
    totcols = idx_bufs.shape[2]
    in_maps = []
    for c in range(N_CORES):
        cpk = np.zeros((128, totcols + 128), dtype=np.int16)
        cpk[:, :totcols] = idx_bufs[c]
        cpk[:, totcols:] = ident
        m = {"cpk": cpk, "trp": trp}
        for b in range(N_BLOCKS):
            m[f"block{b}"] = tabs[b]
        in_maps.append(m)

    if TRACE:
        _ensure_ntff_hook()
        import concourse.bass_utils as _bu

        if not getattr(_bu, "_upload_patched", False):
            _bu.upload_artifacts = lambda d: "local://" + d
            _bu._upload_patched = True
        try:
            res = run_bass_kernel_spmd(
                nc, in_maps, core_ids=list(range(N_CORES)), trace=True
            )
        except Exception:
            res = run_bass_kernel_spmd(
                nc, in_maps, core_ids=list(range(N_CORES)), trace=False
            )
    else:
        res = run_bass_kernel_spmd(
            nc, in_maps, core_ids=list(range(N_CORES)), trace=False
        )
    LAST_EXEC_NS = res.exec_time_ns
    LAST_RESULTS = res

    T = src.size
    out_flat = np.zeros((T, OUT_DIM), dtype=np.float32)
    all_out = np.stack(
        [np.asarray(res.results[c]["out"], dtype=np.float32) for c in range(N_CORES)]
    )
    for b in range(N_BLOCKS):
        toks, inv, urows = binfo[b]
        if urows.size == 0:
            continue
        core = inv // nb[b]
        pos = inv % nb[b]
        out_flat[toks] = all_out[core, ooffs[b] + pos]
    return out_flat.reshape(src.shape + (OUT_DIM,))
